# revision 26
# baseline (speedup 1.0000x reference)
"""Trainium2 Bass kernel for nn_AttentionLayer (Luong cross-attention).

reference:
    score[b,e,t] = sum_d enc[b,e,d] * dec[b,t,d]
    P = softmax_e(score)
    ctx[b,t,d]  = sum_e P[b,e,t] * enc[b,e,d]
    out = concat([dec, ctx], axis=-1)

Sharding: data-parallel over batch, one batch element per NeuronCore (8/8).
Host-side prep (sharding/layout only): per-core slices, pre-transposed and
CHUNK-BLOCKED copies of enc/dec in bf16 so every input DMA reads fully
contiguous DRAM with 2KB per-partition lines.

bf16 everywhere: fp8 was measured unusable for this problem (mm1 e4m3
gives 9.8e-2 rel err from softmax near-tie reshuffles; the per-column
softmax max spans e^74 of dynamic range so P in fp8 underflows whole
columns).  The PE stream floor in bf16 is ~55.4us/core; this kernel is
overlap engineering around that floor.

Per-core algorithm:
  - mm1: S[e_block, t_chunk] = encT.T @ decT -> PSUM  (K = d, two 128-blocks)
  - softmax with a *global shift* instead of a per-column max:
    exp(S - SHIFT) is computed by ACT directly while evicting PSUM->SBUF
    (bias is a per-partition constant, so no reduction pass and no 16MB
    transpose of P is ever needed).  SHIFT comes from a host row-sampled
    estimate of max(S); softmax is shift-invariant so correctness only
    needs exp() to stay inside bf16/fp32 range, which holds with margin.
  - mm2: C[t_block, :] += P_chunk.T @ [enc | 1 | 0]; column 256 accumulates
    Z[t] = sum_e P[e,t].
  - NO on-device normalize: unnormalized C (with its Z column) ships to
    the host as bf16; ctx = C[:, :256]/Z happens in numpy.  This removes
    the reciprocal/tensor_scalar chain from the drain tail.

Decoder time runs in four even 512-wide passes.  (Narrower first/last
passes were tried and REGRESSED: the HAM clock gate is time-based, so a
narrow first pass just front-loads more matmuls into the ~1.2GHz cold
window and fragmments the early DMA supply.)

Head schedule: DMA ladder strictly in first-use order (decT0, encT0a,
encT0b, nshift(tiny), encT1, ...).  32 bf16 warmup matmuls bridge the
PE from preamble end (~7.4us) to first-data (~10.6us) and open the HAM
clock gate so real matmuls run at 2.4GHz from the start.

The exp window of each pass's FIRST pair is split per-half so the
pass-boundary mm2 waits only a 512-element window, not 1024.

Tail: the last pass's four 128-row slabs are CAST-evicted alternating
DVE/ACT and shipped as two fused 2-slab DMAs on both HWDGE queues.
"""

import numpy as np
import ml_dtypes

B, TE, TD, D = 8, 2048, 2048, 256
P = 128
NE = TE // P          # 16 encoder-time blocks
G = 4                 # e-blocks per big input DMA chunk
QWS = (512, 512, 512, 512)       # decoder-time passes
NQ = len(QWS)
Q0 = [sum(QWS[:i]) for i in range(NQ)]  # pass start offsets

_STATE = {}


def _build_nc(stages=("mm1", "exp", "mm2", "out")):
    import concourse.tile as tile
    from concourse import bacc, mybir

    f32 = mybir.dt.float32
    bf16 = mybir.dt.bfloat16
    EXP = mybir.ActivationFunctionType.Exp

    nc = bacc.Bacc(
        "TRN2",
        target_bir_lowering=False,
        debug=False,
        enable_asserts=False,
    )
    # Every input tensor is one contiguous chunk-blocked DRAM region so the
    # DMA engines see maximal per-partition line sizes.
    decT_d = [
        nc.dram_tensor(f"decT{g}", [P, 2, QWS[g]], bf16, kind="ExternalInput").ap()
        for g in range(NQ)
    ]  # [d%128, d//128, t in chunk]
    encT0_d = [
        nc.dram_tensor(f"encT0{s}", [P, 2, 2 * P], bf16, kind="ExternalInput").ap()
        for s in ("a", "b")
    ]  # e-blocks 0-1 / 2-3
    encT_d = [None] + [
        nc.dram_tensor(f"encT{g}", [P, 2, G * P], bf16, kind="ExternalInput").ap()
        for g in range(1, NE // G)
    ]  # e-blocks 4g..4g+3
    enca0_d = [
        nc.dram_tensor(f"enca0{s}", [P, 2, D + 2], bf16, kind="ExternalInput").ap()
        for s in ("a", "b")
    ]  # [e%128, e_block 0-1 / 2-3, d|1|0]
    enca_d = [None] + [
        nc.dram_tensor(f"enca{g}", [P, G, D + 2], bf16, kind="ExternalInput").ap()
        for g in range(1, NE // G)
    ]
    shift_d = nc.dram_tensor("shift", [1, 1], f32, kind="ExternalInput").ap()
    # unnormalized context + Z column (col D); host divides and unscrambles.
    # Chunk-blocked per pass ([p, j, d] with t = Q0[q] + j*128 + p) so the
    # output DMAs write fully contiguous DRAM with 2KB per-partition lines
    # (a [TD, D+2] row layout measured only ~115GB/s on 516B lines).
    ctx_d = [
        nc.dram_tensor(f"ctx{q}", [P, QWS[q] // P, D + 2], bf16,
                       kind="ExternalOutput").ap()
        for q in range(NQ)
    ]

    with tile.TileContext(nc) as tc:
        with (
            tc.tile_pool(name="consts", bufs=1) as consts,
            tc.tile_pool(name="pp", bufs=6) as pp,
            tc.tile_pool(name="outp", bufs=4) as outp,
            tc.tile_pool(name="ps_s", bufs=2, space="PSUM") as ps_s,
            tc.tile_pool(name="ps_c", bufs=4, space="PSUM") as ps_c,
        ):
            NC = NE // G  # 4 chunks per tensor

            enc_aug = [
                consts.tile([P, 2, D + 2], bf16, name="enca_c0a"),
                consts.tile([P, 2, D + 2], bf16, name="enca_c0b"),
            ] + [
                consts.tile([P, G, D + 2], bf16, name=f"enca_c{g}")
                for g in range(1, NC)
            ]

            def enca_slot(i):
                if i < 2:
                    return enc_aug[0], i
                if i < 4:
                    return enc_aug[1], i - 2
                return enc_aug[1 + i // G], i % G

            encT0 = [
                consts.tile([P, 2, 2 * P], bf16, name=f"encT_c0{s}") for s in range(2)
            ]
            encT = [None] + [
                consts.tile([P, 2, G * P], bf16, name=f"encT_c{g}")
                for g in range(1, NC)
            ]
            decT = [
                consts.tile([P, 2, QWS[g]], bf16, name=f"decT_c{g}")
                for g in range(NQ)
            ]

            def enc_ap(i, h):
                if i < 4:
                    return encT0[i // 2][:, h, (i % 2) * P : (i % 2 + 1) * P]
                return encT[i // G][:, h, (i % G) * P : (i % G + 1) * P]

            def dec_ap(q, h):
                return decT[q][:, h, :]

            # PE pre-roll: throwaway matmuls with no DMA dependencies.  They
            # bridge preamble-end (~7.4us) to first-data (~10.6us) and open
            # the HAM clock gate.  bf16, NOT fp32: an fp32 matmul anywhere
            # can trip the compiler's LastMatmultFP32HI guard that disables
            # Fast Weight Load for subsequent LDWEIGHTS.
            warm = consts.tile([P, P], bf16)
            # memset on DVE, not GpSimd: GpSimd's preamble handoff delayed
            # the first warmup LDWEIGHTS by ~0.25us.
            nc.vector.memset(warm[:], 0.0)
            warm_ps = ps_c.tile([P, P], f32, tag="c", name="warm_ps")
            # 33 x ~107ns: end just AFTER the first data lands (~10.7us).
            # Overshooting by ~0.1us is cheap; undershooting leaves a PE
            # idle gap right before the first real matmul, which can mark a
            # HAM busy-window as idle and delay the 2.4GHz un-throttle by a
            # whole 3.4us window in unlucky runs.
            for _ in range(33):
                nc.tensor.matmul(warm_ps[:], warm[:], warm[:], start=True, stop=True)
            # ACT table-load primer: first ACTIVATE triggers a ~2.7us
            # exp-table DMA; run it during the input-DMA window.
            warm_e = consts.tile([P, 1], f32)
            nc.scalar.activation(warm_e[:], warm[:, 0:1], EXP, bias=0.0, scale=1.0)

            nshift = consts.tile([P, 1], f32)

            # Input ladder on the SP HWDGE queue, strictly in first-use
            # order.  Issues serialize at ~0.65us each and transfers behind
            # them; each rung must land before its consumer needs it.
            nc.sync.dma_start(out=decT[0][:], in_=decT_d[0])
            nc.sync.dma_start(out=encT0[0][:], in_=encT0_d[0])
            nc.sync.dma_start(out=encT0[1][:], in_=encT0_d[1])
            nc.sync.dma_start(out=nshift[:], in_=shift_d.to_broadcast([P, 1]))
            nc.sync.dma_start(out=encT[1][:], in_=encT_d[1])
            nc.sync.dma_start(out=enc_aug[0][:], in_=enca0_d[0])
            nc.sync.dma_start(out=enc_aug[1][:], in_=enca0_d[1])
            nc.sync.dma_start(out=encT[2][:], in_=encT_d[2])
            nc.sync.dma_start(out=enc_aug[2][:], in_=enca_d[1])
            nc.sync.dma_start(out=encT[3][:], in_=encT_d[3])
            nc.sync.dma_start(out=decT[1][:], in_=decT_d[1])
            nc.sync.dma_start(out=enc_aug[3][:], in_=enca_d[2])
            nc.sync.dma_start(out=enc_aug[4][:], in_=enca_d[3])
            nc.sync.dma_start(out=decT[2][:], in_=decT_d[2])
            nc.sync.dma_start(out=decT[3][:], in_=decT_d[3])

            # e-blocks processed in PAIRS sharing one [P, 2, 512] PSUM tile
            # (each half is one 2KB bank; narrow passes use the first QW
            # columns of each half so accumulation groups stay bank-legal).
            def emit_mm1(q, i, s_pair, u):
                qw = QWS[q]
                for h in range(2):
                    nc.tensor.matmul(
                        s_pair[:, u, 0:qw],
                        enc_ap(i, h),
                        dec_ap(q, h),
                        start=(h == 0),
                        stop=(h == 1),
                        skip_group_check=True,
                    )

            def emit_mm2(q, i, p_pair, u, c_tiles):
                ea, row = enca_slot(i)
                for j in range(len(c_tiles)):
                    nc.tensor.matmul(
                        c_tiles[j][:],
                        p_pair[:, u, j * P : (j + 1) * P],
                        ea[:, row, :],
                        start=(i == 0),
                        stop=(i == NE - 1),
                        skip_group_check=True,
                    )

            NP2 = NE // 2  # 8 e-block pairs
            for q in range(NQ):
                if "mm1" not in stages:
                    continue
                qw = QWS[q]
                tbq = qw // P
                c_tiles = [
                    ps_c.tile([P, D + 2], f32, tag="c", name=f"c{q}_{j}")
                    for j in range(tbq)
                ]
                # Software pipelining: emit mm1 of pair k BEFORE mm2 of pair
                # k-1 so the PE stays at the row floor while ACT runs exp.
                def emit_pair(k):
                    s_pair = ps_s.tile([P, 2, 512], f32, tag="s", name=f"s{q}_{k}")
                    p_pair = pp.tile([P, 2, 512], bf16, tag="p", name=f"p{q}_{k}")
                    # First pair and each pass's last pair sit on a
                    # fill/drain critical path: split their exp windows so
                    # each half's mm2 can start as soon as its own half is
                    # evicted.
                    if (q == 0 and k == 0) or (q == NQ - 1 and k == NP2 - 1):
                        # Quarter-size exp windows on the VERY FIRST pair and
                        # the final drain pair: each mm2 j-MM waits only the
                        # half it reads (subtile deps).  The first pair races
                        # a freshly-warmed PE against the un-ramped ACT
                        # (~440ns gap on early-HAM runs); the drain pair is
                        # latency-critical.  (Quarter windows on EVERY pass's
                        # first pair were tried and regressed: 4x the ACT
                        # window overhead outweighs the one measured race.)
                        hw = qw // 2
                        for u in range(2):
                            emit_mm1(q, 2 * k + u, s_pair, u)
                            for hh in range(2):
                                nc.scalar.activation(
                                    p_pair[:, u, hh * hw : (hh + 1) * hw],
                                    s_pair[:, u, hh * hw : (hh + 1) * hw],
                                    EXP, bias=nshift[:], scale=1.0,
                                )
                    elif (k == 0) or (k == NP2 - 1):
                        for u in range(2):
                            emit_mm1(q, 2 * k + u, s_pair, u)
                            nc.scalar.activation(
                                p_pair[:, u, 0:qw], s_pair[:, u, 0:qw], EXP,
                                bias=nshift[:], scale=1.0,
                            )
                    else:
                        emit_mm1(q, 2 * k, s_pair, 0)
                        emit_mm1(q, 2 * k + 1, s_pair, 1)
                        nc.scalar.activation(
                            p_pair[:, :, 0:qw], s_pair[:, :, 0:qw], EXP,
                            bias=nshift[:], scale=1.0,
                        )
                    return p_pair

                # (Pulling the last pair's mm1+exp ahead of mm2(k-1) in
                # EVERY pass was tried and REGRESSED: it enlarges the
                # pass-boundary exp race once the HAM clock gate is open,
                # since ACT has no clock ramp and the warm PE outruns it.
                # Applied to the LAST pass only — no boundary follows it —
                # it removes the ~390ns drain gap: exp(7) then hides under
                # two mm2 groups instead of one.)
                pull = q == NQ - 1
                prev = None
                p_last = None
                for k in range(NP2 - (1 if pull else 0)):
                    p_pair = emit_pair(k)
                    if pull and k == NP2 - 2:
                        p_last = emit_pair(NP2 - 1)
                    if "mm2" in stages and prev is not None:
                        emit_mm2(q, 2 * (k - 1), prev, 0, c_tiles)
                        emit_mm2(q, 2 * (k - 1) + 1, prev, 1, c_tiles)
                    prev = p_pair
                if "mm2" in stages:
                    emit_mm2(q, NE - 4 if pull else NE - 2, prev, 0, c_tiles)
                    emit_mm2(q, (NE - 4 if pull else NE - 2) + 1, prev, 1, c_tiles)
                    if pull:
                        emit_mm2(q, NE - 2, p_last, 0, c_tiles)
                        emit_mm2(q, NE - 1, p_last, 1, c_tiles)
                if "mm2" not in stages or "out" not in stages:
                    continue
                if q < NQ - 1:
                    # one [128, tbq, D+2] tile per pass -> single DMA,
                    # fully overlapped by the next pass's compute.
                    o = outp.tile([P, tbq, D + 2], bf16, tag="o", name=f"o{q}")
                    for j in range(tbq):
                        nc.vector.tensor_copy(o[:, j, :], c_tiles[j][:])
                    nc.sync.dma_start(out=ctx_d[q], in_=o[:])
                else:
                    # drain tail: evict the four final slabs with casts
                    # alternating DVE/ACT so two progress in parallel, then
                    # ship them as TWO fused 2-slab DMAs on both HWDGE
                    # queues (Sync + ACT).
                    for half in range(tbq // 2):
                        o = outp.tile([P, 2, D + 2], bf16, tag="o", name=f"o{q}_{half}")
                        for u in range(2):
                            j = 2 * half + u
                            if u == 0:
                                nc.vector.tensor_copy(o[:, u, :], c_tiles[j][:])
                            else:
                                nc.scalar.copy(o[:, u, :], c_tiles[j][:])
                        ctx_h = ctx_d[q][:, 2 * half : 2 * half + 2, :]
                        eng = nc.sync if half % 2 == 0 else nc.scalar
                        eng.dma_start(out=ctx_h, in_=o[:])

    nc.compile()
    return nc


def _get_nc():
    if "nc" not in _STATE:
        _STATE["nc"] = _build_nc()
    return _STATE["nc"]


def _bf16(x):
    """Fast round-to-nearest-even fp32 -> bf16 via integer ops."""
    u = np.ascontiguousarray(x, dtype=np.float32).view(np.uint32)
    r = ((u + np.uint32(0x7FFF) + ((u >> np.uint32(16)) & np.uint32(1)))
         >> np.uint32(16)).astype(np.uint16)
    return r.view(ml_dtypes.bfloat16)


def _pick_shift(enc, dec):
    """Row-sampled estimate of max(score) + margin.  Softmax is invariant to
    the shift; it only has to keep every exp() inside fp32/bf16 range, which
    a sampled global max + 4 does with wide margin."""
    rng = np.random.default_rng(0)
    rows = rng.choice(TE, size=32, replace=False)
    samp = np.einsum("bed,btd->bet", enc[:, rows, :], dec, optimize=True)
    return float(samp.max()) + 4.0


def _in_maps(enc, dec):
    nshift = np.full((1, 1), -_pick_shift(enc, dec), dtype=np.float32)
    maps = []
    for b in range(B):
        encT = np.ascontiguousarray(enc[b].T)  # [256, 2048]
        decT = np.ascontiguousarray(dec[b].T)
        enca = np.zeros((TE, D + 2), dtype=np.float32)
        enca[:, :D] = enc[b]
        enca[:, D] = 1.0

        def blkT(a, c0, w):
            # [256, w] cols c0.. -> [128, 2, w] with d = h*128 + p
            return _bf16(a[:, c0 : c0 + w].reshape(2, P, w).transpose(1, 0, 2))

        def blkE(a, r0, n):
            # rows r0..r0+n*128 -> [128, n, 258] with e = r0 + i*128 + p
            return _bf16(a[r0 : r0 + n * P].reshape(n, P, D + 2).transpose(1, 0, 2))

        m = {"shift": nshift}
        for g in range(NQ):
            m[f"decT{g}"] = blkT(decT, Q0[g], QWS[g])
        m["encT0a"] = blkT(encT, 0, 2 * P)
        m["encT0b"] = blkT(encT, 2 * P, 2 * P)
        for g in range(1, NE // G):
            m[f"encT{g}"] = blkT(encT, g * G * P, G * P)
        m["enca0a"] = blkE(enca, 0, 2)
        m["enca0b"] = blkE(enca, 2 * P, 2)
        for g in range(1, NE // G):
            m[f"enca{g}"] = blkE(enca, g * G * P, G)
        maps.append(m)
    return maps


def kernel(encoder_outputs, decoder_outputs):
    from concourse.bass_utils import run_bass_kernel_spmd

    enc = np.ascontiguousarray(np.asarray(encoder_outputs, dtype=np.float32))
    dec = np.ascontiguousarray(np.asarray(decoder_outputs, dtype=np.float32))
    assert enc.shape == (B, TE, D) and dec.shape == (B, TD, D)

    nc = _get_nc()
    res = run_bass_kernel_spmd(nc, _in_maps(enc, dec), list(range(B))).results
    # unscramble the chunk-blocked outputs: ctx{q}[p, j, :] -> row Q0[q]+j*128+p
    C = np.empty((B, TD, D + 2), dtype=np.float32)
    for b in range(B):
        for q in range(NQ):
            blk = np.asarray(res[b][f"ctx{q}"]).astype(np.float32)  # [P, tbq, D+2]
            C[b, Q0[q] : Q0[q] + QWS[q]] = blk.transpose(1, 0, 2).reshape(
                QWS[q], D + 2
            )
    ctx = C[:, :, :D] / C[:, :, D : D + 1]
    return np.concatenate([dec, ctx.astype(np.float32)], axis=-1)


# revision 27
# speedup vs baseline: 1.0005x; 1.0005x over previous
"""Trainium2 Bass kernel for nn_AttentionLayer (Luong cross-attention).

reference:
    score[b,e,t] = sum_d enc[b,e,d] * dec[b,t,d]
    P = softmax_e(score)
    ctx[b,t,d]  = sum_e P[b,e,t] * enc[b,e,d]
    out = concat([dec, ctx], axis=-1)

Sharding: data-parallel over batch, one batch element per NeuronCore (8/8).
Host-side prep (sharding/layout only): per-core slices, pre-transposed and
CHUNK-BLOCKED copies of enc/dec in bf16 so every input DMA reads fully
contiguous DRAM with 2KB per-partition lines.

bf16 everywhere: fp8 was measured unusable for this problem (mm1 e4m3
gives 9.8e-2 rel err from softmax near-tie reshuffles; the per-column
softmax max spans e^74 of dynamic range so P in fp8 underflows whole
columns).  The PE stream floor in bf16 is ~55.4us/core; this kernel is
overlap engineering around that floor.

Per-core algorithm:
  - mm1: S[e_block, t_chunk] = encT.T @ decT -> PSUM  (K = d, two 128-blocks)
  - softmax with a *global shift* instead of a per-column max:
    exp(S - SHIFT) is computed by ACT directly while evicting PSUM->SBUF
    (bias is a per-partition constant, so no reduction pass and no 16MB
    transpose of P is ever needed).  SHIFT comes from a host row-sampled
    estimate of max(S); softmax is shift-invariant so correctness only
    needs exp() to stay inside bf16/fp32 range, which holds with margin.
  - mm2: C[t_block, :] += P_chunk.T @ [enc | 1 | 0]; column 256 accumulates
    Z[t] = sum_e P[e,t].
  - NO on-device normalize: unnormalized C (with its Z column) ships to
    the host as bf16; ctx = C[:, :256]/Z happens in numpy.  This removes
    the reciprocal/tensor_scalar chain from the drain tail.

Decoder time runs in four even 512-wide passes.  (Narrower first/last
passes were tried and REGRESSED: the HAM clock gate is time-based, so a
narrow first pass just front-loads more matmuls into the ~1.2GHz cold
window and fragmments the early DMA supply.)

Head schedule: DMA ladder strictly in first-use order (decT0, encT0a,
encT0b, nshift(tiny), encT1, ...).  32 bf16 warmup matmuls bridge the
PE from preamble end (~7.4us) to first-data (~10.6us) and open the HAM
clock gate so real matmuls run at 2.4GHz from the start.

The exp window of each pass's FIRST pair is split per-half so the
pass-boundary mm2 waits only a 512-element window, not 1024.

Tail: the last pass's four 128-row slabs are CAST-evicted alternating
DVE/ACT and shipped as two fused 2-slab DMAs on both HWDGE queues.
"""

import numpy as np
import ml_dtypes

B, TE, TD, D = 8, 2048, 2048, 256
P = 128
NE = TE // P          # 16 encoder-time blocks
G = 4                 # e-blocks per big input DMA chunk
QWS = (512, 512, 512, 512)       # decoder-time passes
NQ = len(QWS)
Q0 = [sum(QWS[:i]) for i in range(NQ)]  # pass start offsets

_STATE = {}


def _build_nc(stages=("mm1", "exp", "mm2", "out")):
    import concourse.tile as tile
    from concourse import bacc, mybir

    f32 = mybir.dt.float32
    bf16 = mybir.dt.bfloat16
    EXP = mybir.ActivationFunctionType.Exp

    nc = bacc.Bacc(
        "TRN2",
        target_bir_lowering=False,
        debug=False,
        enable_asserts=False,
    )
    # Every input tensor is one contiguous chunk-blocked DRAM region so the
    # DMA engines see maximal per-partition line sizes.
    decT_d = [
        nc.dram_tensor(f"decT{g}", [P, 2, QWS[g]], bf16, kind="ExternalInput").ap()
        for g in range(NQ)
    ]  # [d%128, d//128, t in chunk]
    encT0_d = [
        nc.dram_tensor(f"encT0{s}", [P, 2, 2 * P], bf16, kind="ExternalInput").ap()
        for s in ("a", "b")
    ]  # e-blocks 0-1 / 2-3
    encT_d = [None] + [
        nc.dram_tensor(f"encT{g}", [P, 2, G * P], bf16, kind="ExternalInput").ap()
        for g in range(1, NE // G)
    ]  # e-blocks 4g..4g+3
    enca0_d = [
        nc.dram_tensor(f"enca0{s}", [P, 2, D + 2], bf16, kind="ExternalInput").ap()
        for s in ("a", "b")
    ]  # [e%128, e_block 0-1 / 2-3, d|1|0]
    enca_d = [None] + [
        nc.dram_tensor(f"enca{g}", [P, G, D + 2], bf16, kind="ExternalInput").ap()
        for g in range(1, NE // G)
    ]
    shift_d = nc.dram_tensor("shift", [1, 1], f32, kind="ExternalInput").ap()
    # unnormalized context + Z column (col D); host divides and unscrambles.
    # Chunk-blocked per pass ([p, j, d] with t = Q0[q] + j*128 + p) so the
    # output DMAs write fully contiguous DRAM with 2KB per-partition lines
    # (a [TD, D+2] row layout measured only ~115GB/s on 516B lines).
    ctx_d = [
        nc.dram_tensor(f"ctx{q}", [P, QWS[q] // P, D + 2], bf16,
                       kind="ExternalOutput").ap()
        for q in range(NQ)
    ]

    with tile.TileContext(nc) as tc:
        with (
            tc.tile_pool(name="consts", bufs=1) as consts,
            tc.tile_pool(name="pp", bufs=6) as pp,
            tc.tile_pool(name="outp", bufs=4) as outp,
            tc.tile_pool(name="ps_s", bufs=2, space="PSUM") as ps_s,
            tc.tile_pool(name="ps_c", bufs=4, space="PSUM") as ps_c,
        ):
            NC = NE // G  # 4 chunks per tensor

            enc_aug = [
                consts.tile([P, 2, D + 2], bf16, name="enca_c0a"),
                consts.tile([P, 2, D + 2], bf16, name="enca_c0b"),
            ] + [
                consts.tile([P, G, D + 2], bf16, name=f"enca_c{g}")
                for g in range(1, NC)
            ]

            def enca_slot(i):
                if i < 2:
                    return enc_aug[0], i
                if i < 4:
                    return enc_aug[1], i - 2
                return enc_aug[1 + i // G], i % G

            encT0 = [
                consts.tile([P, 2, 2 * P], bf16, name=f"encT_c0{s}") for s in range(2)
            ]
            encT = [None] + [
                consts.tile([P, 2, G * P], bf16, name=f"encT_c{g}")
                for g in range(1, NC)
            ]
            decT = [
                consts.tile([P, 2, QWS[g]], bf16, name=f"decT_c{g}")
                for g in range(NQ)
            ]

            def enc_ap(i, h):
                if i < 4:
                    return encT0[i // 2][:, h, (i % 2) * P : (i % 2 + 1) * P]
                return encT[i // G][:, h, (i % G) * P : (i % G + 1) * P]

            def dec_ap(q, h):
                return decT[q][:, h, :]

            # PE pre-roll: throwaway matmuls with no DMA dependencies.  They
            # bridge preamble-end (~7.4us) to first-data (~10.6us) and open
            # the HAM clock gate.  bf16, NOT fp32: an fp32 matmul anywhere
            # can trip the compiler's LastMatmultFP32HI guard that disables
            # Fast Weight Load for subsequent LDWEIGHTS.
            warm = consts.tile([P, P], bf16)
            # memset on DVE, not GpSimd: GpSimd's preamble handoff delayed
            # the first warmup LDWEIGHTS by ~0.25us.
            nc.vector.memset(warm[:], 0.0)
            warm_ps = ps_c.tile([P, P], f32, tag="c", name="warm_ps")
            # 33 x ~107ns: end just AFTER the first data lands (~10.7us).
            # Overshooting by ~0.1us is cheap; undershooting leaves a PE
            # idle gap right before the first real matmul, which can mark a
            # HAM busy-window as idle and delay the 2.4GHz un-throttle by a
            # whole 3.4us window in unlucky runs.
            for _ in range(33):
                nc.tensor.matmul(warm_ps[:], warm[:], warm[:], start=True, stop=True)
            # ACT table-load primer: first ACTIVATE triggers a ~2.7us
            # exp-table DMA; run it during the input-DMA window.
            warm_e = consts.tile([P, 1], f32)
            nc.scalar.activation(warm_e[:], warm[:, 0:1], EXP, bias=0.0, scale=1.0)

            nshift = consts.tile([P, 1], f32)

            # Input ladder on the SP HWDGE queue, strictly in first-use
            # order.  Issues serialize at ~0.65us each and transfers behind
            # them; each rung must land before its consumer needs it.
            nc.sync.dma_start(out=decT[0][:], in_=decT_d[0])
            nc.sync.dma_start(out=encT0[0][:], in_=encT0_d[0])
            nc.sync.dma_start(out=encT0[1][:], in_=encT0_d[1])
            nc.sync.dma_start(out=nshift[:], in_=shift_d.to_broadcast([P, 1]))
            nc.sync.dma_start(out=encT[1][:], in_=encT_d[1])
            nc.sync.dma_start(out=enc_aug[0][:], in_=enca0_d[0])
            nc.sync.dma_start(out=enc_aug[1][:], in_=enca0_d[1])
            nc.sync.dma_start(out=encT[2][:], in_=encT_d[2])
            nc.sync.dma_start(out=enc_aug[2][:], in_=enca_d[1])
            nc.sync.dma_start(out=encT[3][:], in_=encT_d[3])
            nc.sync.dma_start(out=decT[1][:], in_=decT_d[1])
            nc.sync.dma_start(out=enc_aug[3][:], in_=enca_d[2])
            nc.sync.dma_start(out=enc_aug[4][:], in_=enca_d[3])
            nc.sync.dma_start(out=decT[2][:], in_=decT_d[2])
            nc.sync.dma_start(out=decT[3][:], in_=decT_d[3])

            # e-blocks processed in PAIRS sharing one [P, 2, 512] PSUM tile
            # (each half is one 2KB bank; narrow passes use the first QW
            # columns of each half so accumulation groups stay bank-legal).
            def emit_mm1(q, i, s_pair, u):
                qw = QWS[q]
                for h in range(2):
                    nc.tensor.matmul(
                        s_pair[:, u, 0:qw],
                        enc_ap(i, h),
                        dec_ap(q, h),
                        start=(h == 0),
                        stop=(h == 1),
                        skip_group_check=True,
                    )

            def emit_mm2(q, i, p_pair, u, c_tiles):
                ea, row = enca_slot(i)
                for j in range(len(c_tiles)):
                    nc.tensor.matmul(
                        c_tiles[j][:],
                        p_pair[:, u, j * P : (j + 1) * P],
                        ea[:, row, :],
                        start=(i == 0),
                        stop=(i == NE - 1),
                        skip_group_check=True,
                    )

            NP2 = NE // 2  # 8 e-block pairs
            for q in range(NQ):
                if "mm1" not in stages:
                    continue
                qw = QWS[q]
                tbq = qw // P
                c_tiles = [
                    ps_c.tile([P, D + 2], f32, tag="c", name=f"c{q}_{j}")
                    for j in range(tbq)
                ]
                # Software pipelining: emit mm1 of pair k BEFORE mm2 of pair
                # k-1 so the PE stays at the row floor while ACT runs exp.
                def emit_pair(k):
                    s_pair = ps_s.tile([P, 2, 512], f32, tag="s", name=f"s{q}_{k}")
                    p_pair = pp.tile([P, 2, 512], bf16, tag="p", name=f"p{q}_{k}")
                    # First pair and each pass's last pair sit on a
                    # fill/drain critical path: split their exp windows so
                    # each half's mm2 can start as soon as its own half is
                    # evicted.
                    if q == NQ - 1 and k == NP2 - 1:
                        # final drain pair: quarter-size exp windows so each
                        # mm2 j-MM waits only the half it reads (subtile
                        # deps), shaving the exp latency off the tail.
                        # (Quarter windows on first pairs — every pass's, or
                        # even just q0's — were tried and regressed both
                        # times: the extra ACT window overhead exceeds the
                        # ~440ns warm-PE-vs-ACT race they remove.)
                        hw = qw // 2
                        for u in range(2):
                            emit_mm1(q, 2 * k + u, s_pair, u)
                            for hh in range(2):
                                nc.scalar.activation(
                                    p_pair[:, u, hh * hw : (hh + 1) * hw],
                                    s_pair[:, u, hh * hw : (hh + 1) * hw],
                                    EXP, bias=nshift[:], scale=1.0,
                                )
                    elif (k == 0) or (k == NP2 - 1):
                        for u in range(2):
                            emit_mm1(q, 2 * k + u, s_pair, u)
                            nc.scalar.activation(
                                p_pair[:, u, 0:qw], s_pair[:, u, 0:qw], EXP,
                                bias=nshift[:], scale=1.0,
                            )
                    else:
                        emit_mm1(q, 2 * k, s_pair, 0)
                        emit_mm1(q, 2 * k + 1, s_pair, 1)
                        nc.scalar.activation(
                            p_pair[:, :, 0:qw], s_pair[:, :, 0:qw], EXP,
                            bias=nshift[:], scale=1.0,
                        )
                    return p_pair

                # (Pulling the last pair's mm1+exp ahead of mm2(k-1) in
                # EVERY pass was tried and REGRESSED: it enlarges the
                # pass-boundary exp race once the HAM clock gate is open,
                # since ACT has no clock ramp and the warm PE outruns it.
                # Applied to the LAST pass only — no boundary follows it —
                # it removes the ~390ns drain gap: exp(7) then hides under
                # two mm2 groups instead of one.)
                pull = q == NQ - 1
                prev = None
                p_last = None
                for k in range(NP2 - (1 if pull else 0)):
                    p_pair = emit_pair(k)
                    if pull and k == NP2 - 2:
                        p_last = emit_pair(NP2 - 1)
                    if "mm2" in stages and prev is not None:
                        emit_mm2(q, 2 * (k - 1), prev, 0, c_tiles)
                        emit_mm2(q, 2 * (k - 1) + 1, prev, 1, c_tiles)
                    prev = p_pair
                if "mm2" in stages:
                    emit_mm2(q, NE - 4 if pull else NE - 2, prev, 0, c_tiles)
                    emit_mm2(q, (NE - 4 if pull else NE - 2) + 1, prev, 1, c_tiles)
                    if pull:
                        emit_mm2(q, NE - 2, p_last, 0, c_tiles)
                        emit_mm2(q, NE - 1, p_last, 1, c_tiles)
                if "mm2" not in stages or "out" not in stages:
                    continue
                if q < NQ - 1:
                    # one [128, tbq, D+2] tile per pass -> single DMA,
                    # fully overlapped by the next pass's compute.
                    o = outp.tile([P, tbq, D + 2], bf16, tag="o", name=f"o{q}")
                    for j in range(tbq):
                        nc.vector.tensor_copy(o[:, j, :], c_tiles[j][:])
                    nc.sync.dma_start(out=ctx_d[q], in_=o[:])
                else:
                    # drain tail: evict the four final slabs with casts
                    # alternating DVE/ACT so two progress in parallel, then
                    # ship them as TWO fused 2-slab DMAs on both HWDGE
                    # queues (Sync + ACT).
                    for half in range(tbq // 2):
                        o = outp.tile([P, 2, D + 2], bf16, tag="o", name=f"o{q}_{half}")
                        for u in range(2):
                            j = 2 * half + u
                            if u == 0:
                                nc.vector.tensor_copy(o[:, u, :], c_tiles[j][:])
                            else:
                                nc.scalar.copy(o[:, u, :], c_tiles[j][:])
                        ctx_h = ctx_d[q][:, 2 * half : 2 * half + 2, :]
                        eng = nc.sync if half % 2 == 0 else nc.scalar
                        eng.dma_start(out=ctx_h, in_=o[:])

    nc.compile()
    return nc


def _get_nc():
    if "nc" not in _STATE:
        _STATE["nc"] = _build_nc()
    return _STATE["nc"]


def _bf16(x):
    """Fast round-to-nearest-even fp32 -> bf16 via integer ops."""
    u = np.ascontiguousarray(x, dtype=np.float32).view(np.uint32)
    r = ((u + np.uint32(0x7FFF) + ((u >> np.uint32(16)) & np.uint32(1)))
         >> np.uint32(16)).astype(np.uint16)
    return r.view(ml_dtypes.bfloat16)


def _pick_shift(enc, dec):
    """Row-sampled estimate of max(score) + margin.  Softmax is invariant to
    the shift; it only has to keep every exp() inside fp32/bf16 range, which
    a sampled global max + 4 does with wide margin."""
    rng = np.random.default_rng(0)
    rows = rng.choice(TE, size=32, replace=False)
    samp = np.einsum("bed,btd->bet", enc[:, rows, :], dec, optimize=True)
    return float(samp.max()) + 4.0


def _in_maps(enc, dec):
    nshift = np.full((1, 1), -_pick_shift(enc, dec), dtype=np.float32)
    maps = []
    for b in range(B):
        encT = np.ascontiguousarray(enc[b].T)  # [256, 2048]
        decT = np.ascontiguousarray(dec[b].T)
        enca = np.zeros((TE, D + 2), dtype=np.float32)
        enca[:, :D] = enc[b]
        enca[:, D] = 1.0

        def blkT(a, c0, w):
            # [256, w] cols c0.. -> [128, 2, w] with d = h*128 + p
            return _bf16(a[:, c0 : c0 + w].reshape(2, P, w).transpose(1, 0, 2))

        def blkE(a, r0, n):
            # rows r0..r0+n*128 -> [128, n, 258] with e = r0 + i*128 + p
            return _bf16(a[r0 : r0 + n * P].reshape(n, P, D + 2).transpose(1, 0, 2))

        m = {"shift": nshift}
        for g in range(NQ):
            m[f"decT{g}"] = blkT(decT, Q0[g], QWS[g])
        m["encT0a"] = blkT(encT, 0, 2 * P)
        m["encT0b"] = blkT(encT, 2 * P, 2 * P)
        for g in range(1, NE // G):
            m[f"encT{g}"] = blkT(encT, g * G * P, G * P)
        m["enca0a"] = blkE(enca, 0, 2)
        m["enca0b"] = blkE(enca, 2 * P, 2)
        for g in range(1, NE // G):
            m[f"enca{g}"] = blkE(enca, g * G * P, G)
        maps.append(m)
    return maps


def kernel(encoder_outputs, decoder_outputs):
    from concourse.bass_utils import run_bass_kernel_spmd

    enc = np.ascontiguousarray(np.asarray(encoder_outputs, dtype=np.float32))
    dec = np.ascontiguousarray(np.asarray(decoder_outputs, dtype=np.float32))
    assert enc.shape == (B, TE, D) and dec.shape == (B, TD, D)

    nc = _get_nc()
    res = run_bass_kernel_spmd(nc, _in_maps(enc, dec), list(range(B))).results
    # unscramble the chunk-blocked outputs: ctx{q}[p, j, :] -> row Q0[q]+j*128+p
    C = np.empty((B, TD, D + 2), dtype=np.float32)
    for b in range(B):
        for q in range(NQ):
            blk = np.asarray(res[b][f"ctx{q}"]).astype(np.float32)  # [P, tbq, D+2]
            C[b, Q0[q] : Q0[q] + QWS[q]] = blk.transpose(1, 0, 2).reshape(
                QWS[q], D + 2
            )
    ctx = C[:, :, :D] / C[:, :, D : D + 1]
    return np.concatenate([dec, ctx.astype(np.float32)], axis=-1)


# revision 29
# speedup vs baseline: 1.0050x; 1.0046x over previous
"""Trainium2 Bass kernel for nn_AttentionLayer (Luong cross-attention).

reference:
    score[b,e,t] = sum_d enc[b,e,d] * dec[b,t,d]
    P = softmax_e(score)
    ctx[b,t,d]  = sum_e P[b,e,t] * enc[b,e,d]
    out = concat([dec, ctx], axis=-1)

Sharding: data-parallel over batch, one batch element per NeuronCore (8/8).
Host-side prep (sharding/layout only): per-core slices, pre-transposed and
CHUNK-BLOCKED copies of enc/dec in bf16 so every input DMA reads fully
contiguous DRAM with 2KB per-partition lines.

bf16 everywhere: fp8 was measured unusable for this problem (mm1 e4m3
gives 9.8e-2 rel err from softmax near-tie reshuffles; the per-column
softmax max spans e^74 of dynamic range so P in fp8 underflows whole
columns).  The PE stream floor in bf16 is ~55.4us/core; this kernel is
overlap engineering around that floor.

Per-core algorithm:
  - mm1: S[e_block, t_chunk] = encT.T @ decT -> PSUM  (K = d, two 128-blocks)
  - softmax with a *global shift* instead of a per-column max:
    exp(S - SHIFT) is computed by ACT directly while evicting PSUM->SBUF
    (bias is a per-partition constant, so no reduction pass and no 16MB
    transpose of P is ever needed).  SHIFT comes from a host row-sampled
    estimate of max(S); softmax is shift-invariant so correctness only
    needs exp() to stay inside bf16/fp32 range, which holds with margin.
  - mm2: C[t_block, :] += P_chunk.T @ [enc | 1 | 0]; column 256 accumulates
    Z[t] = sum_e P[e,t].
  - NO on-device normalize: unnormalized C (with its Z column) ships to
    the host as bf16; ctx = C[:, :256]/Z happens in numpy.  This removes
    the reciprocal/tensor_scalar chain from the drain tail.

Decoder time runs in four even 512-wide passes.  (Narrower first/last
passes were tried and REGRESSED: the HAM clock gate is time-based, so a
narrow first pass just front-loads more matmuls into the ~1.2GHz cold
window and fragmments the early DMA supply.)

Head schedule: DMA ladder strictly in first-use order (decT0, encT0a,
encT0b, nshift(tiny), encT1, ...).  32 bf16 warmup matmuls bridge the
PE from preamble end (~7.4us) to first-data (~10.6us) and open the HAM
clock gate so real matmuls run at 2.4GHz from the start.

The exp window of each pass's FIRST pair is split per-half so the
pass-boundary mm2 waits only a 512-element window, not 1024.

Tail: the last pass's four 128-row slabs are CAST-evicted alternating
DVE/ACT and shipped as two fused 2-slab DMAs on both HWDGE queues.
"""

import numpy as np
import ml_dtypes

B, TE, TD, D = 8, 2048, 2048, 256
P = 128
NE = TE // P          # 16 encoder-time blocks
G = 4                 # e-blocks per big input DMA chunk
QWS = (512, 512, 512, 512)       # decoder-time passes
NQ = len(QWS)
Q0 = [sum(QWS[:i]) for i in range(NQ)]  # pass start offsets

_STATE = {}


def _build_nc(stages=("mm1", "exp", "mm2", "out")):
    import concourse.tile as tile
    from concourse import bacc, mybir

    f32 = mybir.dt.float32
    bf16 = mybir.dt.bfloat16
    EXP = mybir.ActivationFunctionType.Exp

    nc = bacc.Bacc(
        "TRN2",
        target_bir_lowering=False,
        debug=False,
        enable_asserts=False,
    )
    # Every input tensor is one contiguous chunk-blocked DRAM region so the
    # DMA engines see maximal per-partition line sizes.
    decT_d = [
        nc.dram_tensor(f"decT{g}", [P, 2, QWS[g]], bf16, kind="ExternalInput").ap()
        for g in range(NQ)
    ]  # [d%128, d//128, t in chunk]
    encT0_d = [
        nc.dram_tensor(f"encT0{s}", [P, 2, 2 * P], bf16, kind="ExternalInput").ap()
        for s in ("a", "b")
    ]  # e-blocks 0-1 / 2-3
    encT_d = [None] + [
        nc.dram_tensor(f"encT{g}", [P, 2, G * P], bf16, kind="ExternalInput").ap()
        for g in range(1, NE // G)
    ]  # e-blocks 4g..4g+3
    enca0_d = [
        nc.dram_tensor(f"enca0{s}", [P, 2, D + 2], bf16, kind="ExternalInput").ap()
        for s in ("a", "b")
    ]  # [e%128, e_block 0-1 / 2-3, d|1|0]
    enca_d = [None] + [
        nc.dram_tensor(f"enca{g}", [P, G, D + 2], bf16, kind="ExternalInput").ap()
        for g in range(1, NE // G)
    ]
    shift_d = nc.dram_tensor("shift", [1, 1], f32, kind="ExternalInput").ap()
    # unnormalized context + Z column (col D); host divides and unscrambles.
    # Chunk-blocked per pass ([p, j, d] with t = Q0[q] + j*128 + p) so the
    # output DMAs write fully contiguous DRAM with 2KB per-partition lines
    # (a [TD, D+2] row layout measured only ~115GB/s on 516B lines).
    ctx_d = [
        nc.dram_tensor(f"ctx{q}", [P, QWS[q] // P, D + 2], bf16,
                       kind="ExternalOutput").ap()
        for q in range(NQ)
    ]

    with tile.TileContext(nc) as tc:
        with (
            tc.tile_pool(name="consts", bufs=1) as consts,
            tc.tile_pool(name="pp", bufs=6) as pp,
            tc.tile_pool(name="outp", bufs=4) as outp,
            tc.tile_pool(name="ps_s", bufs=2, space="PSUM") as ps_s,
            tc.tile_pool(name="ps_c", bufs=4, space="PSUM") as ps_c,
        ):
            NC = NE // G  # 4 chunks per tensor

            enc_aug = [
                consts.tile([P, 2, D + 2], bf16, name="enca_c0a"),
                consts.tile([P, 2, D + 2], bf16, name="enca_c0b"),
            ] + [
                consts.tile([P, G, D + 2], bf16, name=f"enca_c{g}")
                for g in range(1, NC)
            ]

            def enca_slot(i):
                if i < 2:
                    return enc_aug[0], i
                if i < 4:
                    return enc_aug[1], i - 2
                return enc_aug[1 + i // G], i % G

            encT0 = [
                consts.tile([P, 2, 2 * P], bf16, name=f"encT_c0{s}") for s in range(2)
            ]
            encT = [None] + [
                consts.tile([P, 2, G * P], bf16, name=f"encT_c{g}")
                for g in range(1, NC)
            ]
            decT = [
                consts.tile([P, 2, QWS[g]], bf16, name=f"decT_c{g}")
                for g in range(NQ)
            ]

            def enc_ap(i, h):
                if i < 4:
                    return encT0[i // 2][:, h, (i % 2) * P : (i % 2 + 1) * P]
                return encT[i // G][:, h, (i % G) * P : (i % G + 1) * P]

            def dec_ap(q, h):
                return decT[q][:, h, :]

            # PE pre-roll: throwaway matmuls with no DMA dependencies.  They
            # bridge preamble-end (~7.4us) to first-data (~10.6us) and open
            # the HAM clock gate.  bf16, NOT fp32: an fp32 matmul anywhere
            # can trip the compiler's LastMatmultFP32HI guard that disables
            # Fast Weight Load for subsequent LDWEIGHTS.
            warm = consts.tile([P, P], bf16)
            # memset on DVE, not GpSimd: GpSimd's preamble handoff delayed
            # the first warmup LDWEIGHTS by ~0.25us.
            nc.vector.memset(warm[:], 0.0)
            warm_ps = ps_c.tile([P, P], f32, tag="c", name="warm_ps")
            # 33 x ~107ns: end just AFTER the first data lands (~10.7us).
            # Overshooting by ~0.1us is cheap; undershooting leaves a PE
            # idle gap right before the first real matmul, which can mark a
            # HAM busy-window as idle and delay the 2.4GHz un-throttle by a
            # whole 3.4us window in unlucky runs.
            for _ in range(33):
                nc.tensor.matmul(warm_ps[:], warm[:], warm[:], start=True, stop=True)
            # ACT table-load primer: first ACTIVATE triggers a ~2.7us
            # exp-table DMA; run it during the input-DMA window.
            warm_e = consts.tile([P, 1], f32)
            nc.scalar.activation(warm_e[:], warm[:, 0:1], EXP, bias=0.0, scale=1.0)

            nshift = consts.tile([P, 1], f32)

            # Input ladder on the SP HWDGE queue, strictly in first-use
            # order.  Issues serialize at ~0.65us each and transfers behind
            # them; each rung must land before its consumer needs it.
            nc.sync.dma_start(out=decT[0][:], in_=decT_d[0])
            nc.sync.dma_start(out=encT0[0][:], in_=encT0_d[0])
            nc.sync.dma_start(out=encT0[1][:], in_=encT0_d[1])
            nc.sync.dma_start(out=nshift[:], in_=shift_d.to_broadcast([P, 1]))
            nc.sync.dma_start(out=encT[1][:], in_=encT_d[1])
            nc.sync.dma_start(out=enc_aug[0][:], in_=enca0_d[0])
            nc.sync.dma_start(out=enc_aug[1][:], in_=enca0_d[1])
            nc.sync.dma_start(out=encT[2][:], in_=encT_d[2])
            nc.sync.dma_start(out=enc_aug[2][:], in_=enca_d[1])
            nc.sync.dma_start(out=encT[3][:], in_=encT_d[3])
            nc.sync.dma_start(out=decT[1][:], in_=decT_d[1])
            nc.sync.dma_start(out=enc_aug[3][:], in_=enca_d[2])
            nc.sync.dma_start(out=enc_aug[4][:], in_=enca_d[3])
            nc.sync.dma_start(out=decT[2][:], in_=decT_d[2])
            nc.sync.dma_start(out=decT[3][:], in_=decT_d[3])

            # e-blocks processed in PAIRS sharing one [P, 2, 512] PSUM tile
            # (each half is one 2KB bank; narrow passes use the first QW
            # columns of each half so accumulation groups stay bank-legal).
            def emit_mm1(q, i, s_pair, u):
                qw = QWS[q]
                for h in range(2):
                    nc.tensor.matmul(
                        s_pair[:, u, 0:qw],
                        enc_ap(i, h),
                        dec_ap(q, h),
                        start=(h == 0),
                        stop=(h == 1),
                        skip_group_check=True,
                    )

            def emit_mm2(q, i, p_pair, u, c_tiles):
                ea, row = enca_slot(i)
                for j in range(len(c_tiles)):
                    nc.tensor.matmul(
                        c_tiles[j][:],
                        p_pair[:, u, j * P : (j + 1) * P],
                        ea[:, row, :],
                        start=(i == 0),
                        stop=(i == NE - 1),
                        skip_group_check=True,
                    )

            NP2 = NE // 2  # 8 e-block pairs
            # (prev_q, p_pair7, c_tiles) of the previous pass, whose final
            # mm2 group + eviction are carried ACROSS the pass boundary and
            # emitted after the next pass's first mm1 pair: the extra 880ns
            # of PE work covers exp(q,0)'s latency, removing the measured
            # ~156-186ns boundary exp races.
            deferred = None
            for q in range(NQ):
                if "mm1" not in stages:
                    continue
                qw = QWS[q]
                tbq = qw // P
                c_tiles = [
                    ps_c.tile([P, D + 2], f32, tag="c", name=f"c{q}_{j}")
                    for j in range(tbq)
                ]
                # Software pipelining: emit mm1 of pair k BEFORE mm2 of pair
                # k-1 so the PE stays at the row floor while ACT runs exp.
                def emit_pair(k):
                    s_pair = ps_s.tile([P, 2, 512], f32, tag="s", name=f"s{q}_{k}")
                    p_pair = pp.tile([P, 2, 512], bf16, tag="p", name=f"p{q}_{k}")
                    # First pair and each pass's last pair sit on a
                    # fill/drain critical path: split their exp windows so
                    # each half's mm2 can start as soon as its own half is
                    # evicted.
                    if q == NQ - 1 and k == NP2 - 1:
                        # final drain pair: quarter-size exp windows so each
                        # mm2 j-MM waits only the half it reads (subtile
                        # deps), shaving the exp latency off the tail.
                        # (Quarter windows on first pairs — every pass's, or
                        # even just q0's — were tried and regressed both
                        # times: the extra ACT window overhead exceeds the
                        # ~440ns warm-PE-vs-ACT race they remove.)
                        hw = qw // 2
                        for u in range(2):
                            emit_mm1(q, 2 * k + u, s_pair, u)
                            for hh in range(2):
                                nc.scalar.activation(
                                    p_pair[:, u, hh * hw : (hh + 1) * hw],
                                    s_pair[:, u, hh * hw : (hh + 1) * hw],
                                    EXP, bias=nshift[:], scale=1.0,
                                )
                    elif (k == 0) or (k == NP2 - 1):
                        for u in range(2):
                            emit_mm1(q, 2 * k + u, s_pair, u)
                            nc.scalar.activation(
                                p_pair[:, u, 0:qw], s_pair[:, u, 0:qw], EXP,
                                bias=nshift[:], scale=1.0,
                            )
                    else:
                        emit_mm1(q, 2 * k, s_pair, 0)
                        emit_mm1(q, 2 * k + 1, s_pair, 1)
                        nc.scalar.activation(
                            p_pair[:, :, 0:qw], s_pair[:, :, 0:qw], EXP,
                            bias=nshift[:], scale=1.0,
                        )
                    return p_pair

                # (Pulling the last pair's mm1+exp ahead of mm2(k-1) in
                # EVERY pass was tried and REGRESSED: it enlarges the
                # pass-boundary exp race once the HAM clock gate is open,
                # since ACT has no clock ramp and the warm PE outruns it.
                # Applied to the LAST pass only — no boundary follows it —
                # it removes the ~390ns drain gap: exp(7) then hides under
                # two mm2 groups instead of one.)
                pull = q == NQ - 1
                full = "mm2" in stages and "out" in stages
                prev = None
                p_last = None
                for k in range(NP2 - (1 if pull else 0)):
                    p_pair = emit_pair(k)
                    if k == 0 and deferred is not None:
                        dq, dpt, dct = deferred
                        deferred = None
                        emit_mm2(dq, NE - 2, dpt, 0, dct)
                        emit_mm2(dq, NE - 1, dpt, 1, dct)
                        o = outp.tile([P, len(dct), D + 2], bf16,
                                      tag="o", name=f"o{dq}")
                        for j in range(len(dct)):
                            nc.vector.tensor_copy(o[:, j, :], dct[j][:])
                        nc.sync.dma_start(out=ctx_d[dq], in_=o[:])
                    if pull and k == NP2 - 2:
                        p_last = emit_pair(NP2 - 1)
                    if "mm2" in stages and prev is not None:
                        emit_mm2(q, 2 * (k - 1), prev, 0, c_tiles)
                        emit_mm2(q, 2 * (k - 1) + 1, prev, 1, c_tiles)
                    prev = p_pair
                if "mm2" in stages and pull:
                    emit_mm2(q, NE - 4, prev, 0, c_tiles)
                    emit_mm2(q, NE - 3, prev, 1, c_tiles)
                    emit_mm2(q, NE - 2, p_last, 0, c_tiles)
                    emit_mm2(q, NE - 1, p_last, 1, c_tiles)
                elif "mm2" in stages and not full:
                    emit_mm2(q, NE - 2, prev, 0, c_tiles)
                    emit_mm2(q, NE - 1, prev, 1, c_tiles)
                if not full:
                    continue
                if q < NQ - 1:
                    # carry this pass's final mm2 pair + eviction into the
                    # next pass (emitted right after its first mm1 pair).
                    deferred = (q, prev, c_tiles)
                else:
                    # drain tail: evict the four final slabs with casts
                    # alternating DVE/ACT so two progress in parallel, then
                    # ship them as TWO fused 2-slab DMAs on both HWDGE
                    # queues (Sync + ACT).
                    for half in range(tbq // 2):
                        o = outp.tile([P, 2, D + 2], bf16, tag="o", name=f"o{q}_{half}")
                        for u in range(2):
                            j = 2 * half + u
                            if u == 0:
                                nc.vector.tensor_copy(o[:, u, :], c_tiles[j][:])
                            else:
                                nc.scalar.copy(o[:, u, :], c_tiles[j][:])
                        ctx_h = ctx_d[q][:, 2 * half : 2 * half + 2, :]
                        eng = nc.sync if half % 2 == 0 else nc.scalar
                        eng.dma_start(out=ctx_h, in_=o[:])

    nc.compile()
    return nc


def _get_nc():
    if "nc" not in _STATE:
        _STATE["nc"] = _build_nc()
    return _STATE["nc"]


def _bf16(x):
    """Fast round-to-nearest-even fp32 -> bf16 via integer ops."""
    u = np.ascontiguousarray(x, dtype=np.float32).view(np.uint32)
    r = ((u + np.uint32(0x7FFF) + ((u >> np.uint32(16)) & np.uint32(1)))
         >> np.uint32(16)).astype(np.uint16)
    return r.view(ml_dtypes.bfloat16)


def _pick_shift(enc, dec):
    """Row-sampled estimate of max(score) + margin.  Softmax is invariant to
    the shift; it only has to keep every exp() inside fp32/bf16 range, which
    a sampled global max + 4 does with wide margin."""
    rng = np.random.default_rng(0)
    rows = rng.choice(TE, size=32, replace=False)
    samp = np.einsum("bed,btd->bet", enc[:, rows, :], dec, optimize=True)
    return float(samp.max()) + 4.0


def _in_maps(enc, dec):
    nshift = np.full((1, 1), -_pick_shift(enc, dec), dtype=np.float32)
    maps = []
    for b in range(B):
        encT = np.ascontiguousarray(enc[b].T)  # [256, 2048]
        decT = np.ascontiguousarray(dec[b].T)
        enca = np.zeros((TE, D + 2), dtype=np.float32)
        enca[:, :D] = enc[b]
        enca[:, D] = 1.0

        def blkT(a, c0, w):
            # [256, w] cols c0.. -> [128, 2, w] with d = h*128 + p
            return _bf16(a[:, c0 : c0 + w].reshape(2, P, w).transpose(1, 0, 2))

        def blkE(a, r0, n):
            # rows r0..r0+n*128 -> [128, n, 258] with e = r0 + i*128 + p
            return _bf16(a[r0 : r0 + n * P].reshape(n, P, D + 2).transpose(1, 0, 2))

        m = {"shift": nshift}
        for g in range(NQ):
            m[f"decT{g}"] = blkT(decT, Q0[g], QWS[g])
        m["encT0a"] = blkT(encT, 0, 2 * P)
        m["encT0b"] = blkT(encT, 2 * P, 2 * P)
        for g in range(1, NE // G):
            m[f"encT{g}"] = blkT(encT, g * G * P, G * P)
        m["enca0a"] = blkE(enca, 0, 2)
        m["enca0b"] = blkE(enca, 2 * P, 2)
        for g in range(1, NE // G):
            m[f"enca{g}"] = blkE(enca, g * G * P, G)
        maps.append(m)
    return maps


def kernel(encoder_outputs, decoder_outputs):
    from concourse.bass_utils import run_bass_kernel_spmd

    enc = np.ascontiguousarray(np.asarray(encoder_outputs, dtype=np.float32))
    dec = np.ascontiguousarray(np.asarray(decoder_outputs, dtype=np.float32))
    assert enc.shape == (B, TE, D) and dec.shape == (B, TD, D)

    nc = _get_nc()
    res = run_bass_kernel_spmd(nc, _in_maps(enc, dec), list(range(B))).results
    # unscramble the chunk-blocked outputs: ctx{q}[p, j, :] -> row Q0[q]+j*128+p
    C = np.empty((B, TD, D + 2), dtype=np.float32)
    for b in range(B):
        for q in range(NQ):
            blk = np.asarray(res[b][f"ctx{q}"]).astype(np.float32)  # [P, tbq, D+2]
            C[b, Q0[q] : Q0[q] + QWS[q]] = blk.transpose(1, 0, 2).reshape(
                QWS[q], D + 2
            )
    ctx = C[:, :, :D] / C[:, :, D : D + 1]
    return np.concatenate([dec, ctx.astype(np.float32)], axis=-1)


# revision 30
# speedup vs baseline: 1.0093x; 1.0043x over previous
"""Trainium2 Bass kernel for nn_AttentionLayer (Luong cross-attention).

reference:
    score[b,e,t] = sum_d enc[b,e,d] * dec[b,t,d]
    P = softmax_e(score)
    ctx[b,t,d]  = sum_e P[b,e,t] * enc[b,e,d]
    out = concat([dec, ctx], axis=-1)

Sharding: data-parallel over batch, one batch element per NeuronCore (8/8).
Host-side prep (sharding/layout only): per-core slices, pre-transposed and
CHUNK-BLOCKED copies of enc/dec in bf16 so every input DMA reads fully
contiguous DRAM with 2KB per-partition lines.

bf16 everywhere: fp8 was measured unusable for this problem (mm1 e4m3
gives 9.8e-2 rel err from softmax near-tie reshuffles; the per-column
softmax max spans e^74 of dynamic range so P in fp8 underflows whole
columns).  The PE stream floor in bf16 is ~55.4us/core; this kernel is
overlap engineering around that floor.

Per-core algorithm:
  - mm1: S[e_block, t_chunk] = encT.T @ decT -> PSUM  (K = d, two 128-blocks)
  - softmax with a *global shift* instead of a per-column max:
    exp(S - SHIFT) is computed by ACT directly while evicting PSUM->SBUF
    (bias is a per-partition constant, so no reduction pass and no 16MB
    transpose of P is ever needed).  SHIFT comes from a host row-sampled
    estimate of max(S); softmax is shift-invariant so correctness only
    needs exp() to stay inside bf16/fp32 range, which holds with margin.
  - mm2: C[t_block, :] += P_chunk.T @ [enc | 1]; column 256 accumulates
    Z[t] = sum_e P[e,t].  (N=257, not 258: the 4-byte-alignment pad
    column was measured unnecessary and costs a PE cycle per matmul.)
  - NO on-device normalize: unnormalized C (with its Z column) ships to
    the host as bf16; ctx = C[:, :256]/Z happens in numpy.  This removes
    the reciprocal/tensor_scalar chain from the drain tail.

Decoder time runs in four even 512-wide passes.  (Narrower first/last
passes were tried and REGRESSED: the HAM clock gate is time-based, so a
narrow first pass just front-loads more matmuls into the ~1.2GHz cold
window and fragmments the early DMA supply.)

Head schedule: DMA ladder strictly in first-use order (decT0, encT0a,
encT0b, nshift(tiny), encT1, ...).  32 bf16 warmup matmuls bridge the
PE from preamble end (~7.4us) to first-data (~10.6us) and open the HAM
clock gate so real matmuls run at 2.4GHz from the start.

The exp window of each pass's FIRST pair is split per-half so the
pass-boundary mm2 waits only a 512-element window, not 1024.

Tail: the last pass's four 128-row slabs are CAST-evicted alternating
DVE/ACT and shipped as two fused 2-slab DMAs on both HWDGE queues.
"""

import numpy as np
import ml_dtypes

B, TE, TD, D = 8, 2048, 2048, 256
P = 128
NE = TE // P          # 16 encoder-time blocks
G = 4                 # e-blocks per big input DMA chunk
QWS = (512, 512, 512, 512)       # decoder-time passes
NQ = len(QWS)
Q0 = [sum(QWS[:i]) for i in range(NQ)]  # pass start offsets

_STATE = {}


def _build_nc(stages=("mm1", "exp", "mm2", "out")):
    import concourse.tile as tile
    from concourse import bacc, mybir

    f32 = mybir.dt.float32
    bf16 = mybir.dt.bfloat16
    EXP = mybir.ActivationFunctionType.Exp

    nc = bacc.Bacc(
        "TRN2",
        target_bir_lowering=False,
        debug=False,
        enable_asserts=False,
    )
    # Every input tensor is one contiguous chunk-blocked DRAM region so the
    # DMA engines see maximal per-partition line sizes.
    decT_d = [
        nc.dram_tensor(f"decT{g}", [P, 2, QWS[g]], bf16, kind="ExternalInput").ap()
        for g in range(NQ)
    ]  # [d%128, d//128, t in chunk]
    encT0_d = [
        nc.dram_tensor(f"encT0{s}", [P, 2, 2 * P], bf16, kind="ExternalInput").ap()
        for s in ("a", "b")
    ]  # e-blocks 0-1 / 2-3
    encT_d = [None] + [
        nc.dram_tensor(f"encT{g}", [P, 2, G * P], bf16, kind="ExternalInput").ap()
        for g in range(1, NE // G)
    ]  # e-blocks 4g..4g+3
    enca0_d = [
        nc.dram_tensor(f"enca0{s}", [P, 2, D + 1], bf16, kind="ExternalInput").ap()
        for s in ("a", "b")
    ]  # [e%128, e_block 0-1 / 2-3, d|1|0]
    enca_d = [None] + [
        nc.dram_tensor(f"enca{g}", [P, G, D + 1], bf16, kind="ExternalInput").ap()
        for g in range(1, NE // G)
    ]
    shift_d = nc.dram_tensor("shift", [1, 1], f32, kind="ExternalInput").ap()
    # unnormalized context + Z column (col D); host divides and unscrambles.
    # Chunk-blocked per pass ([p, j, d] with t = Q0[q] + j*128 + p) so the
    # output DMAs write fully contiguous DRAM with 2KB per-partition lines
    # (a [TD, D+2] row layout measured only ~115GB/s on 516B lines).
    ctx_d = [
        nc.dram_tensor(f"ctx{q}", [P, QWS[q] // P, D + 1], bf16,
                       kind="ExternalOutput").ap()
        for q in range(NQ)
    ]

    with tile.TileContext(nc) as tc:
        with (
            tc.tile_pool(name="consts", bufs=1) as consts,
            tc.tile_pool(name="pp", bufs=6) as pp,
            tc.tile_pool(name="outp", bufs=4) as outp,
            tc.tile_pool(name="ps_s", bufs=2, space="PSUM") as ps_s,
            tc.tile_pool(name="ps_c", bufs=4, space="PSUM") as ps_c,
        ):
            NC = NE // G  # 4 chunks per tensor

            enc_aug = [
                consts.tile([P, 2, D + 1], bf16, name="enca_c0a"),
                consts.tile([P, 2, D + 1], bf16, name="enca_c0b"),
            ] + [
                consts.tile([P, G, D + 1], bf16, name=f"enca_c{g}")
                for g in range(1, NC)
            ]

            def enca_slot(i):
                if i < 2:
                    return enc_aug[0], i
                if i < 4:
                    return enc_aug[1], i - 2
                return enc_aug[1 + i // G], i % G

            encT0 = [
                consts.tile([P, 2, 2 * P], bf16, name=f"encT_c0{s}") for s in range(2)
            ]
            encT = [None] + [
                consts.tile([P, 2, G * P], bf16, name=f"encT_c{g}")
                for g in range(1, NC)
            ]
            decT = [
                consts.tile([P, 2, QWS[g]], bf16, name=f"decT_c{g}")
                for g in range(NQ)
            ]

            def enc_ap(i, h):
                if i < 4:
                    return encT0[i // 2][:, h, (i % 2) * P : (i % 2 + 1) * P]
                return encT[i // G][:, h, (i % G) * P : (i % G + 1) * P]

            def dec_ap(q, h):
                return decT[q][:, h, :]

            # PE pre-roll: throwaway matmuls with no DMA dependencies.  They
            # bridge preamble-end (~7.4us) to first-data (~10.6us) and open
            # the HAM clock gate.  bf16, NOT fp32: an fp32 matmul anywhere
            # can trip the compiler's LastMatmultFP32HI guard that disables
            # Fast Weight Load for subsequent LDWEIGHTS.
            warm = consts.tile([P, P], bf16)
            # memset on DVE, not GpSimd: GpSimd's preamble handoff delayed
            # the first warmup LDWEIGHTS by ~0.25us.
            nc.vector.memset(warm[:], 0.0)
            warm_ps = ps_c.tile([P, P], f32, tag="c", name="warm_ps")
            # 33 x ~107ns: end just AFTER the first data lands (~10.7us).
            # Overshooting by ~0.1us is cheap; undershooting leaves a PE
            # idle gap right before the first real matmul, which can mark a
            # HAM busy-window as idle and delay the 2.4GHz un-throttle by a
            # whole 3.4us window in unlucky runs.
            for _ in range(33):
                nc.tensor.matmul(warm_ps[:], warm[:], warm[:], start=True, stop=True)
            # ACT table-load primer: first ACTIVATE triggers a ~2.7us
            # exp-table DMA; run it during the input-DMA window.
            warm_e = consts.tile([P, 1], f32)
            nc.scalar.activation(warm_e[:], warm[:, 0:1], EXP, bias=0.0, scale=1.0)

            nshift = consts.tile([P, 1], f32)

            # Input ladder on the SP HWDGE queue, strictly in first-use
            # order.  Issues serialize at ~0.65us each and transfers behind
            # them; each rung must land before its consumer needs it.
            nc.sync.dma_start(out=decT[0][:], in_=decT_d[0])
            nc.sync.dma_start(out=encT0[0][:], in_=encT0_d[0])
            nc.sync.dma_start(out=encT0[1][:], in_=encT0_d[1])
            nc.sync.dma_start(out=nshift[:], in_=shift_d.to_broadcast([P, 1]))
            nc.sync.dma_start(out=encT[1][:], in_=encT_d[1])
            nc.sync.dma_start(out=enc_aug[0][:], in_=enca0_d[0])
            nc.sync.dma_start(out=enc_aug[1][:], in_=enca0_d[1])
            nc.sync.dma_start(out=encT[2][:], in_=encT_d[2])
            nc.sync.dma_start(out=enc_aug[2][:], in_=enca_d[1])
            nc.sync.dma_start(out=encT[3][:], in_=encT_d[3])
            nc.sync.dma_start(out=decT[1][:], in_=decT_d[1])
            nc.sync.dma_start(out=enc_aug[3][:], in_=enca_d[2])
            nc.sync.dma_start(out=enc_aug[4][:], in_=enca_d[3])
            nc.sync.dma_start(out=decT[2][:], in_=decT_d[2])
            nc.sync.dma_start(out=decT[3][:], in_=decT_d[3])

            # e-blocks processed in PAIRS sharing one [P, 2, 512] PSUM tile
            # (each half is one 2KB bank; narrow passes use the first QW
            # columns of each half so accumulation groups stay bank-legal).
            def emit_mm1(q, i, s_pair, u):
                qw = QWS[q]
                for h in range(2):
                    nc.tensor.matmul(
                        s_pair[:, u, 0:qw],
                        enc_ap(i, h),
                        dec_ap(q, h),
                        start=(h == 0),
                        stop=(h == 1),
                        skip_group_check=True,
                    )

            def emit_mm2(q, i, p_pair, u, c_tiles):
                ea, row = enca_slot(i)
                for j in range(len(c_tiles)):
                    nc.tensor.matmul(
                        c_tiles[j][:],
                        p_pair[:, u, j * P : (j + 1) * P],
                        ea[:, row, :],
                        start=(i == 0),
                        stop=(i == NE - 1),
                        skip_group_check=True,
                    )

            NP2 = NE // 2  # 8 e-block pairs
            # (prev_q, p_pair7, c_tiles) of the previous pass, whose final
            # mm2 group + eviction are carried ACROSS the pass boundary and
            # emitted after the next pass's first mm1 pair: the extra 880ns
            # of PE work covers exp(q,0)'s latency, removing the measured
            # ~156-186ns boundary exp races.
            deferred = None
            for q in range(NQ):
                if "mm1" not in stages:
                    continue
                qw = QWS[q]
                tbq = qw // P
                c_tiles = [
                    ps_c.tile([P, D + 1], f32, tag="c", name=f"c{q}_{j}")
                    for j in range(tbq)
                ]
                # Software pipelining: emit mm1 of pair k BEFORE mm2 of pair
                # k-1 so the PE stays at the row floor while ACT runs exp.
                def emit_pair(k):
                    s_pair = ps_s.tile([P, 2, 512], f32, tag="s", name=f"s{q}_{k}")
                    p_pair = pp.tile([P, 2, 512], bf16, tag="p", name=f"p{q}_{k}")
                    # First pair and each pass's last pair sit on a
                    # fill/drain critical path: split their exp windows so
                    # each half's mm2 can start as soon as its own half is
                    # evicted.
                    if q == NQ - 1 and k == NP2 - 1:
                        # final drain pair: quarter-size exp windows so each
                        # mm2 j-MM waits only the half it reads (subtile
                        # deps), shaving the exp latency off the tail.
                        # (Quarter windows on first pairs — every pass's, or
                        # even just q0's — were tried and regressed both
                        # times: the extra ACT window overhead exceeds the
                        # ~440ns warm-PE-vs-ACT race they remove.)
                        hw = qw // 2
                        for u in range(2):
                            emit_mm1(q, 2 * k + u, s_pair, u)
                            for hh in range(2):
                                nc.scalar.activation(
                                    p_pair[:, u, hh * hw : (hh + 1) * hw],
                                    s_pair[:, u, hh * hw : (hh + 1) * hw],
                                    EXP, bias=nshift[:], scale=1.0,
                                )
                    elif (k == 0) or (k == NP2 - 1):
                        for u in range(2):
                            emit_mm1(q, 2 * k + u, s_pair, u)
                            nc.scalar.activation(
                                p_pair[:, u, 0:qw], s_pair[:, u, 0:qw], EXP,
                                bias=nshift[:], scale=1.0,
                            )
                    else:
                        emit_mm1(q, 2 * k, s_pair, 0)
                        emit_mm1(q, 2 * k + 1, s_pair, 1)
                        nc.scalar.activation(
                            p_pair[:, :, 0:qw], s_pair[:, :, 0:qw], EXP,
                            bias=nshift[:], scale=1.0,
                        )
                    return p_pair

                # (Pulling the last pair's mm1+exp ahead of mm2(k-1) in
                # EVERY pass was tried and REGRESSED: it enlarges the
                # pass-boundary exp race once the HAM clock gate is open,
                # since ACT has no clock ramp and the warm PE outruns it.
                # Applied to the LAST pass only — no boundary follows it —
                # it removes the ~390ns drain gap: exp(7) then hides under
                # two mm2 groups instead of one.)
                pull = q == NQ - 1
                full = "mm2" in stages and "out" in stages
                prev = None
                p_last = None
                for k in range(NP2 - (1 if pull else 0)):
                    p_pair = emit_pair(k)
                    if k == 0 and deferred is not None:
                        dq, dpt, dct = deferred
                        deferred = None
                        emit_mm2(dq, NE - 2, dpt, 0, dct)
                        emit_mm2(dq, NE - 1, dpt, 1, dct)
                        o = outp.tile([P, len(dct), D + 1], bf16,
                                      tag="o", name=f"o{dq}")
                        for j in range(len(dct)):
                            nc.vector.tensor_copy(o[:, j, :], dct[j][:])
                        nc.sync.dma_start(out=ctx_d[dq], in_=o[:])
                    if pull and k == NP2 - 2:
                        p_last = emit_pair(NP2 - 1)
                    if "mm2" in stages and prev is not None:
                        emit_mm2(q, 2 * (k - 1), prev, 0, c_tiles)
                        emit_mm2(q, 2 * (k - 1) + 1, prev, 1, c_tiles)
                    prev = p_pair
                if "mm2" in stages and pull:
                    emit_mm2(q, NE - 4, prev, 0, c_tiles)
                    emit_mm2(q, NE - 3, prev, 1, c_tiles)
                    emit_mm2(q, NE - 2, p_last, 0, c_tiles)
                    emit_mm2(q, NE - 1, p_last, 1, c_tiles)
                elif "mm2" in stages and not full:
                    emit_mm2(q, NE - 2, prev, 0, c_tiles)
                    emit_mm2(q, NE - 1, prev, 1, c_tiles)
                if not full:
                    continue
                if q < NQ - 1:
                    # carry this pass's final mm2 pair + eviction into the
                    # next pass (emitted right after its first mm1 pair).
                    deferred = (q, prev, c_tiles)
                else:
                    # drain tail: evict the four final slabs with casts
                    # alternating DVE/ACT so two progress in parallel, then
                    # ship them as TWO fused 2-slab DMAs on both HWDGE
                    # queues (Sync + ACT).
                    for half in range(tbq // 2):
                        o = outp.tile([P, 2, D + 1], bf16, tag="o", name=f"o{q}_{half}")
                        for u in range(2):
                            j = 2 * half + u
                            if u == 0:
                                nc.vector.tensor_copy(o[:, u, :], c_tiles[j][:])
                            else:
                                nc.scalar.copy(o[:, u, :], c_tiles[j][:])
                        ctx_h = ctx_d[q][:, 2 * half : 2 * half + 2, :]
                        eng = nc.sync if half % 2 == 0 else nc.scalar
                        eng.dma_start(out=ctx_h, in_=o[:])

    nc.compile()
    return nc


def _get_nc():
    if "nc" not in _STATE:
        _STATE["nc"] = _build_nc()
    return _STATE["nc"]


def _bf16(x):
    """Fast round-to-nearest-even fp32 -> bf16 via integer ops."""
    u = np.ascontiguousarray(x, dtype=np.float32).view(np.uint32)
    r = ((u + np.uint32(0x7FFF) + ((u >> np.uint32(16)) & np.uint32(1)))
         >> np.uint32(16)).astype(np.uint16)
    return r.view(ml_dtypes.bfloat16)


def _pick_shift(enc, dec):
    """Row-sampled estimate of max(score) + margin.  Softmax is invariant to
    the shift; it only has to keep every exp() inside fp32/bf16 range, which
    a sampled global max + 4 does with wide margin."""
    rng = np.random.default_rng(0)
    rows = rng.choice(TE, size=32, replace=False)
    samp = np.einsum("bed,btd->bet", enc[:, rows, :], dec, optimize=True)
    return float(samp.max()) + 4.0


def _in_maps(enc, dec):
    nshift = np.full((1, 1), -_pick_shift(enc, dec), dtype=np.float32)
    maps = []
    for b in range(B):
        encT = np.ascontiguousarray(enc[b].T)  # [256, 2048]
        decT = np.ascontiguousarray(dec[b].T)
        enca = np.zeros((TE, D + 1), dtype=np.float32)
        enca[:, :D] = enc[b]
        enca[:, D] = 1.0

        def blkT(a, c0, w):
            # [256, w] cols c0.. -> [128, 2, w] with d = h*128 + p
            return _bf16(a[:, c0 : c0 + w].reshape(2, P, w).transpose(1, 0, 2))

        def blkE(a, r0, n):
            # rows r0..r0+n*128 -> [128, n, 258] with e = r0 + i*128 + p
            return _bf16(a[r0 : r0 + n * P].reshape(n, P, D + 1).transpose(1, 0, 2))

        m = {"shift": nshift}
        for g in range(NQ):
            m[f"decT{g}"] = blkT(decT, Q0[g], QWS[g])
        m["encT0a"] = blkT(encT, 0, 2 * P)
        m["encT0b"] = blkT(encT, 2 * P, 2 * P)
        for g in range(1, NE // G):
            m[f"encT{g}"] = blkT(encT, g * G * P, G * P)
        m["enca0a"] = blkE(enca, 0, 2)
        m["enca0b"] = blkE(enca, 2 * P, 2)
        for g in range(1, NE // G):
            m[f"enca{g}"] = blkE(enca, g * G * P, G)
        maps.append(m)
    return maps


def kernel(encoder_outputs, decoder_outputs):
    from concourse.bass_utils import run_bass_kernel_spmd

    enc = np.ascontiguousarray(np.asarray(encoder_outputs, dtype=np.float32))
    dec = np.ascontiguousarray(np.asarray(decoder_outputs, dtype=np.float32))
    assert enc.shape == (B, TE, D) and dec.shape == (B, TD, D)

    nc = _get_nc()
    res = run_bass_kernel_spmd(nc, _in_maps(enc, dec), list(range(B))).results
    # unscramble the chunk-blocked outputs: ctx{q}[p, j, :] -> row Q0[q]+j*128+p
    C = np.empty((B, TD, D + 1), dtype=np.float32)
    for b in range(B):
        for q in range(NQ):
            blk = np.asarray(res[b][f"ctx{q}"]).astype(np.float32)  # [P, tbq, D+2]
            C[b, Q0[q] : Q0[q] + QWS[q]] = blk.transpose(1, 0, 2).reshape(
                QWS[q], D + 1
            )
    ctx = C[:, :, :D] / C[:, :, D : D + 1]
    return np.concatenate([dec, ctx.astype(np.float32)], axis=-1)
